# revision 4
# baseline (speedup 1.0000x reference)
"""Multi-head causal attention (B=4, T=2048, D=1024, H=16) on 8 TRN2 cores.

Tensor-parallel over heads: core c computes heads {2c, 2c+1}. Each core:
  - QK' feature-major ([feat, tok]) and V token-major via PE matmuls,
  - S^T = K'^T Q' tiles [128 k x 512 q] (so softmax reductions over keys
    happen via a PE ones-column and P^T is directly the PV moving operand),
  - exp (no max subtraction: |S|/32 <= ~2), multiplicative 0/1 causal mask,
  - PV with V augmented by a ones column (fused denominator),
  - per-query normalization via rank-1 PE broadcast + DVE multiply,
  - w_proj row-slice partial matmul, written feature-major.
Host sums the 8 partial projections and transposes back.
"""

import sys

for _p in ("/opt/trn_rl_repo",):
    if _p not in sys.path:
        sys.path.append(_p)

import numpy as np
import ml_dtypes

B, T, D = 4, 2048, 1024
H = 16
HD = D // H
NORM = float(np.sqrt(D))
N_CORES = 8
HEADS_PER_CORE = H // N_CORES          # 2
FPC = HEADS_PER_CORE * HD              # 128 features per core
QC = 512                               # query chunk
NQC = T // QC                          # 4
KB = 128                               # key block
DKC = D // 128                         # 8 contraction chunks over D

_BF16 = ml_dtypes.bfloat16

_cache = {}


def _build():
    import concourse.bacc as bacc
    import concourse.mybir as mybir
    from concourse.tile import TileContext
    from concourse.alu_op_type import AluOpType

    f32 = mybir.dt.float32
    bf16 = mybir.dt.bfloat16
    EXP = mybir.ActivationFunctionType.Exp

    nc = bacc.Bacc("TRN2", target_bir_lowering=False, debug=False,
                   num_devices=N_CORES)

    xt = nc.dram_tensor("xt", [B, D, T], bf16, kind="ExternalInput").ap()
    wqk = nc.dram_tensor("wqk", [D, 2 * FPC], bf16, kind="ExternalInput").ap()
    wv = nc.dram_tensor("wv", [D, FPC], bf16, kind="ExternalInput").ap()
    wp = nc.dram_tensor("wp", [FPC, D], bf16, kind="ExternalInput").ap()
    masks = nc.dram_tensor("masks", [4, KB, QC], bf16, kind="ExternalInput").ap()
    out = nc.dram_tensor("out", [B, D, T], f32, kind="ExternalOutput").ap()

    with TileContext(nc) as tc:
        with (
            tc.tile_pool(name="const", bufs=1) as cpool,
            tc.tile_pool(name="xp", bufs=10) as xpool,
            tc.tile_pool(name="qk", bufs=2) as qkpool,
            tc.tile_pool(name="vaug", bufs=20) as vpool,
            tc.tile_pool(name="pt", bufs=20) as ptpool,
            tc.tile_pool(name="y", bufs=2) as ypool,
            tc.tile_pool(name="sm", bufs=4) as smpool,
            tc.tile_pool(name="ot", bufs=4) as otpool,
            tc.tile_pool(name="psA", bufs=3, space="PSUM") as psA,
            tc.tile_pool(name="psB", bufs=1, space="PSUM") as psB,
            tc.tile_pool(name="psY", bufs=2, space="PSUM") as psY,
            tc.tile_pool(name="psO", bufs=2, space="PSUM") as psO,
        ):
            # ---- constants ----
            wqk_t = []
            for kc in range(DKC):
                t = cpool.tile([128, 2 * FPC], bf16, tag=f"wqk{kc}")
                nc.sync.dma_start(t[:], wqk[kc * 128:(kc + 1) * 128, :])
                wqk_t.append(t)
            wv_t = []
            for kc in range(DKC):
                t = cpool.tile([128, FPC], bf16, tag=f"wv{kc}")
                nc.sync.dma_start(t[:], wv[kc * 128:(kc + 1) * 128, :])
                wv_t.append(t)
            wp_t = cpool.tile([FPC, D], bf16, tag="wp")
            nc.sync.dma_start(wp_t[:], wp[:])
            mask_t = []
            for p in range(4):
                t = cpool.tile([KB, QC], bf16, tag=f"mask{p}")
                nc.sync.dma_start(t[:], masks[p])
                mask_t.append(t)
            ones_col = cpool.tile([128, 1], bf16, tag="ones_col")
            nc.vector.memset(ones_col[:], 1.0)
            ones_row = cpool.tile([1, 64], f32, tag="ones_row")
            nc.vector.memset(ones_row[:], 1.0)

            for b in range(B):
                # ---- load x^T (feature-major) ----
                xp_t = []
                for kc in range(DKC):
                    t = xpool.tile([128, T], bf16, tag="xp")
                    nc.sync.dma_start(t[:], xt[b, kc * 128:(kc + 1) * 128, :])
                    xp_t.append(t)

                # ---- Q', K' feature-major [128, T] ----
                qp = qkpool.tile([128, T], bf16, tag="qp")
                kp = qkpool.tile([128, T], bf16, tag="kp")
                for half, dst in ((0, qp), (1, kp)):
                    for ntk in range(NQC):
                        ps = psA.tile([128, QC], f32, tag="ps")
                        for kc in range(DKC):
                            nc.tensor.matmul(
                                ps[:],
                                lhsT=wqk_t[kc][:, 128 * half:128 * (half + 1)],
                                rhs=xp_t[kc][:, QC * ntk:QC * (ntk + 1)],
                                start=(kc == 0), stop=(kc == DKC - 1),
                            )
                        nc.vector.tensor_copy(dst[:, QC * ntk:QC * (ntk + 1)], ps[:])

                # ---- V token-major, augmented with ones cols ----
                # layout: [v_h0(64) | ones | v_h1(64) | ones]
                vaug_t = []
                for tk in range(T // 128):
                    ps = psB.tile([128, FPC], f32, tag="psv")
                    for kc in range(DKC):
                        nc.tensor.matmul(
                            ps[:],
                            lhsT=xp_t[kc][:, 128 * tk:128 * (tk + 1)],
                            rhs=wv_t[kc][:],
                            start=(kc == 0), stop=(kc == DKC - 1),
                        )
                    va = vpool.tile([128, 2 * HD + 2], bf16, tag="vaug")
                    nc.vector.tensor_copy(va[:, 0:HD], ps[:, 0:HD])
                    nc.vector.tensor_copy(va[:, HD + 1:2 * HD + 1], ps[:, HD:2 * HD])
                    nc.vector.tensor_copy(va[:, HD:HD + 1], ones_col[:])
                    nc.vector.tensor_copy(va[:, 2 * HD + 1:2 * HD + 2], ones_col[:])
                    vaug_t.append(va)

                # ---- attention + projection per query chunk ----
                for qc in range(NQC):
                    y = ypool.tile([FPC, QC], bf16, tag="y")
                    for h in range(HEADS_PER_CORE):
                        nkb = (qc + 1) * (QC // KB)
                        pts = []
                        for kb in range(nkb):
                            pss = psA.tile([128, QC], f32, tag="ps")
                            nc.tensor.matmul(
                                pss[:],
                                lhsT=kp[HD * h:HD * (h + 1), KB * kb:KB * (kb + 1)],
                                rhs=qp[HD * h:HD * (h + 1), QC * qc:QC * (qc + 1)],
                                start=True, stop=True,
                            )
                            pt = ptpool.tile([KB, QC], bf16, tag="pt")
                            nc.scalar.activation(pt[:], pss[:], EXP, scale=1.0 / NORM)
                            if kb >= qc * 4:
                                nc.vector.tensor_tensor(
                                    pt[:], pt[:], mask_t[kb - qc * 4][:],
                                    op=AluOpType.mult,
                                )
                            pts.append(pt)
                        psy = psY.tile([HD + 1, QC], f32, tag="psy")
                        for kb in range(nkb):
                            nc.tensor.matmul(
                                psy[0:HD + 1, :],
                                lhsT=vaug_t[kb][:, (HD + 1) * h:(HD + 1) * (h + 1)],
                                rhs=pts[kb][:],
                                start=(kb == 0), stop=(kb == nkb - 1),
                            )
                        recip = smpool.tile([1, QC], f32, tag="recip")
                        nc.vector.reciprocal(recip[:], psy[HD:HD + 1, :])
                        psb = psA.tile([64, QC], f32, tag="ps")
                        nc.tensor.matmul(psb[:], lhsT=ones_row[:], rhs=recip[:],
                                         start=True, stop=True)
                        bc = smpool.tile([64, QC], f32, tag="bc")
                        nc.vector.tensor_copy(bc[:], psb[:])
                        nc.vector.tensor_tensor(
                            y[HD * h:HD * (h + 1), :], psy[0:HD, :], bc[:],
                            op=AluOpType.mult,
                        )
                    for mt in range(D // 128):
                        pso = psO.tile([128, QC], f32, tag="pso")
                        nc.tensor.matmul(
                            pso[:],
                            lhsT=wp_t[:, 128 * mt:128 * (mt + 1)],
                            rhs=y[:],
                            start=True, stop=True,
                        )
                        ot = otpool.tile([128, QC], f32, tag="ot")
                        nc.vector.tensor_copy(ot[:], pso[:])
                        nc.sync.dma_start(
                            out[b, 128 * mt:128 * (mt + 1), QC * qc:QC * (qc + 1)],
                            ot[:],
                        )

    nc.compile()
    return nc


def _get_nc():
    if "nc" not in _cache:
        _cache["nc"] = _build()
    return _cache["nc"]


def _make_masks():
    i = np.arange(KB)[:, None]
    j = np.arange(QC)[None, :]
    m = np.zeros((4, KB, QC), dtype=np.float32)
    for p in range(4):
        m[p] = (j >= (KB * p + i)).astype(np.float32)
    return m.astype(_BF16)


def shard_inputs(x, w_qkv, w_proj):
    xt = np.ascontiguousarray(np.asarray(x, dtype=np.float32).transpose(0, 2, 1))
    xt = xt.astype(_BF16)
    w_qkv = np.asarray(w_qkv, dtype=np.float32)
    w_proj = np.asarray(w_proj, dtype=np.float32)
    masks = _make_masks()
    in_maps = []
    for c in range(N_CORES):
        qcols = slice(FPC * c, FPC * (c + 1))
        kcols = slice(D + FPC * c, D + FPC * (c + 1))
        vcols = slice(2 * D + FPC * c, 2 * D + FPC * (c + 1))
        wqk_c = np.concatenate([w_qkv[:, qcols], w_qkv[:, kcols]], axis=1)
        in_maps.append({
            "xt": xt,
            "wqk": np.ascontiguousarray(wqk_c).astype(_BF16),
            "wv": np.ascontiguousarray(w_qkv[:, vcols]).astype(_BF16),
            "wp": np.ascontiguousarray(w_proj[FPC * c:FPC * (c + 1), :]).astype(_BF16),
            "masks": masks,
        })
    return in_maps


def unshard(results):
    total = results[0]["out"].astype(np.float32)
    for r in results[1:]:
        total += r["out"]
    return np.ascontiguousarray(total.transpose(0, 2, 1))


def run(inputs, trace=False, **kw):
    from concourse.bass_utils import run_bass_kernel_spmd

    nc = _get_nc()
    in_maps = shard_inputs(inputs["x"], inputs["w_qkv"], inputs["w_proj"])
    res = run_bass_kernel_spmd(nc, in_maps, core_ids=list(range(N_CORES)),
                               trace=trace, **kw)
    return unshard(res.results), res


def kernel(**inputs):
    out, _ = run(inputs, trace=False)
    return out


# revision 6
# speedup vs baseline: 1.3231x; 1.3231x over previous
"""Multi-head causal attention (B=4, T=2048, D=1024, H=16) on 8 TRN2 cores.

Tensor-parallel over heads: core c computes heads {2c, 2c+1}. Each core:
  - Q', K', V' feature-major ([feat, tok]) via 512-wide PE matmuls,
  - V' -> token-major V via PE transposes, augmented with a ones column
    (fused softmax denominator),
  - S^T = K'^T Q' tiles [128 k x 512 q] (so softmax reductions over keys
    happen on the PE and P^T is directly the PV moving operand),
  - exp (no max subtraction: |S|/32 <= ~2), multiplicative 0/1 causal mask,
    diagonal blocks narrowed to their live query range,
  - per-query normalization via DVE reciprocal + GpSimd partition broadcast,
  - w_proj row-slice partial matmul, written feature-major.
Host sums the 8 partial projections and transposes back.
"""

import sys

for _p in ("/opt/trn_rl_repo",):
    if _p not in sys.path:
        sys.path.append(_p)

import numpy as np
import ml_dtypes

B, T, D = 4, 2048, 1024
H = 16
HD = D // H
NORM = float(np.sqrt(D))
N_CORES = 8
HEADS_PER_CORE = H // N_CORES          # 2
FPC = HEADS_PER_CORE * HD              # 128 features per core
QC = 512                               # query chunk
NQC = T // QC                          # 4
KB = 128                               # key block
DKC = D // 128                         # 8 contraction chunks over D

_BF16 = ml_dtypes.bfloat16

_cache = {}


def _build():
    import concourse.bacc as bacc
    import concourse.mybir as mybir
    from concourse.tile import TileContext
    from concourse.alu_op_type import AluOpType
    from concourse.masks import make_identity

    f32 = mybir.dt.float32
    bf16 = mybir.dt.bfloat16
    EXP = mybir.ActivationFunctionType.Exp

    nc = bacc.Bacc("TRN2", target_bir_lowering=False, debug=False,
                   num_devices=N_CORES)

    xt = nc.dram_tensor("xt", [B, D, T], bf16, kind="ExternalInput").ap()
    w3 = nc.dram_tensor("w3", [D, 3 * FPC], bf16, kind="ExternalInput").ap()
    wp = nc.dram_tensor("wp", [FPC, D], bf16, kind="ExternalInput").ap()
    masks = nc.dram_tensor("masks", [4, KB, QC], bf16, kind="ExternalInput").ap()
    out = nc.dram_tensor("out", [B, D, T], f32, kind="ExternalOutput").ap()

    with TileContext(nc) as tc:
        with (
            tc.tile_pool(name="const", bufs=1) as cpool,
            tc.tile_pool(name="xp", bufs=16) as xpool,
            tc.tile_pool(name="qk", bufs=2) as qkpool,
            tc.tile_pool(name="vaug", bufs=20) as vpool,
            tc.tile_pool(name="pt", bufs=36) as ptpool,
            tc.tile_pool(name="y", bufs=2) as ypool,
            tc.tile_pool(name="sm", bufs=4) as smpool,
            tc.tile_pool(name="ot", bufs=4) as otpool,
            tc.tile_pool(name="psA", bufs=3, space="PSUM") as psA,
            tc.tile_pool(name="psB", bufs=1, space="PSUM") as psB,
            tc.tile_pool(name="psY", bufs=2, space="PSUM") as psY,
            tc.tile_pool(name="psO", bufs=2, space="PSUM") as psO,
        ):
            # ---- constants ----
            w3_t = []
            for kc in range(DKC):
                t = cpool.tile([128, 3 * FPC], bf16, tag=f"w3{kc}")
                nc.sync.dma_start(t[:], w3[kc * 128:(kc + 1) * 128, :])
                w3_t.append(t)
            wp_t = cpool.tile([FPC, D], bf16, tag="wp")
            nc.sync.dma_start(wp_t[:], wp[:])
            mask_t = []
            for p in range(4):
                t = cpool.tile([KB, QC], bf16, tag=f"mask{p}")
                nc.sync.dma_start(t[:], masks[p])
                mask_t.append(t)
            ones_col = cpool.tile([128, 1], bf16, tag="ones_col")
            nc.vector.memset(ones_col[:], 1.0)
            ident = cpool.tile([128, 128], bf16, tag="ident")
            make_identity(nc, ident[:])

            for b in range(B):
                # ---- load x^T (feature-major) ----
                xp_t = []
                for kc in range(DKC):
                    t = xpool.tile([128, T], bf16, tag="xp")
                    nc.sync.dma_start(t[:], xt[b, kc * 128:(kc + 1) * 128, :])
                    xp_t.append(t)

                # ---- Q', K', V' feature-major [128, T] ----
                with nc.named_scope("qkv"):
                    qp = qkpool.tile([128, T], bf16, tag="qp")
                    kp = qkpool.tile([128, T], bf16, tag="kp")
                    vp = qkpool.tile([128, T], bf16, tag="vp")
                    for ft, dst in ((0, qp), (1, kp), (2, vp)):
                        for ntk in range(NQC):
                            ps = psA.tile([128, QC], f32, tag="ps")
                            for kc in range(DKC):
                                nc.tensor.matmul(
                                    ps[:],
                                    lhsT=w3_t[kc][:, 128 * ft:128 * (ft + 1)],
                                    rhs=xp_t[kc][:, QC * ntk:QC * (ntk + 1)],
                                    start=(kc == 0), stop=(kc == DKC - 1),
                                )
                            nc.vector.tensor_copy(dst[:, QC * ntk:QC * (ntk + 1)], ps[:])

                # ---- V' -> token-major V, augmented with ones cols ----
                # layout: [v_h0(64) | ones | v_h1(64) | ones]
                with nc.named_scope("vtrans"):
                    vaug_t = []
                    for tk in range(T // 128):
                        ps = psB.tile([128, FPC], bf16, tag="psv")
                        nc.tensor.transpose(
                            ps[:], vp[:, 128 * tk:128 * (tk + 1)], ident[:]
                        )
                        va = vpool.tile([128, 2 * HD + 2], bf16, tag="vaug")
                        nc.vector.tensor_copy(va[:, 0:HD], ps[:, 0:HD])
                        nc.vector.tensor_copy(va[:, HD + 1:2 * HD + 1], ps[:, HD:2 * HD])
                        nc.vector.tensor_copy(va[:, HD:HD + 1], ones_col[:])
                        nc.vector.tensor_copy(va[:, 2 * HD + 1:2 * HD + 2], ones_col[:])
                        vaug_t.append(va)

                # ---- attention + projection per query chunk ----
                for qc in range(NQC):
                    nkb = (qc + 1) * (QC // KB)
                    y = ypool.tile([FPC, QC], bf16, tag="y")
                    pts = {}
                    with nc.named_scope("score"):
                        for h in range(HEADS_PER_CORE):
                            for kb in range(nkb):
                                p = kb - qc * 4
                                j0 = KB * p if p > 0 else 0
                                pss = psA.tile([128, QC], f32, tag="ps")
                                nc.tensor.matmul(
                                    pss[:, j0:QC],
                                    lhsT=kp[HD * h:HD * (h + 1), KB * kb:KB * (kb + 1)],
                                    rhs=qp[HD * h:HD * (h + 1), QC * qc + j0:QC * (qc + 1)],
                                    start=True, stop=True,
                                )
                                pt = ptpool.tile([KB, QC], bf16, tag="pt")
                                nc.scalar.activation(pt[:, j0:QC], pss[:, j0:QC],
                                                     EXP, scale=1.0 / NORM)
                                if p >= 0:
                                    nc.vector.tensor_tensor(
                                        pt[:, j0:QC], pt[:, j0:QC],
                                        mask_t[p][:, j0:QC],
                                        op=AluOpType.mult,
                                    )
                                pts[h, kb] = (pt, j0)
                    with nc.named_scope("pv"):
                        for h in range(HEADS_PER_CORE):
                            psy = psY.tile([HD + 1, QC], f32, tag="psy")
                            for kb in range(nkb):
                                pt, j0 = pts[h, kb]
                                nc.tensor.matmul(
                                    psy[0:HD + 1, j0:QC],
                                    lhsT=vaug_t[kb][:, (HD + 1) * h:(HD + 1) * (h + 1)],
                                    rhs=pt[:, j0:QC],
                                    start=(kb == 0), stop=(kb == nkb - 1),
                                )
                            recip = smpool.tile([1, QC], f32, tag="recip")
                            nc.vector.reciprocal(recip[:], psy[HD:HD + 1, :])
                            bc = smpool.tile([64, QC], f32, tag="bc")
                            nc.gpsimd.partition_broadcast(bc[:], recip[:])
                            nc.vector.tensor_tensor(
                                y[HD * h:HD * (h + 1), :], psy[0:HD, :], bc[:],
                                op=AluOpType.mult,
                            )
                    with nc.named_scope("proj"):
                        for mt in range(D // 128):
                            pso = psO.tile([128, QC], f32, tag="pso")
                            nc.tensor.matmul(
                                pso[:],
                                lhsT=wp_t[:, 128 * mt:128 * (mt + 1)],
                                rhs=y[:],
                                start=True, stop=True,
                            )
                            ot = otpool.tile([128, QC], f32, tag="ot")
                            nc.vector.tensor_copy(ot[:], pso[:])
                            nc.sync.dma_start(
                                out[b, 128 * mt:128 * (mt + 1), QC * qc:QC * (qc + 1)],
                                ot[:],
                            )

    nc.compile()
    return nc


def _get_nc():
    if "nc" not in _cache:
        _cache["nc"] = _build()
    return _cache["nc"]


def _make_masks():
    i = np.arange(KB)[:, None]
    j = np.arange(QC)[None, :]
    m = np.zeros((4, KB, QC), dtype=np.float32)
    for p in range(4):
        m[p] = (j >= (KB * p + i)).astype(np.float32)
    return m.astype(_BF16)


def shard_inputs(x, w_qkv, w_proj):
    xt = np.ascontiguousarray(np.asarray(x, dtype=np.float32).transpose(0, 2, 1))
    xt = xt.astype(_BF16)
    w_qkv = np.asarray(w_qkv, dtype=np.float32)
    w_proj = np.asarray(w_proj, dtype=np.float32)
    masks = _make_masks()
    in_maps = []
    for c in range(N_CORES):
        qcols = slice(FPC * c, FPC * (c + 1))
        kcols = slice(D + FPC * c, D + FPC * (c + 1))
        vcols = slice(2 * D + FPC * c, 2 * D + FPC * (c + 1))
        w3_c = np.concatenate(
            [w_qkv[:, qcols], w_qkv[:, kcols], w_qkv[:, vcols]], axis=1)
        in_maps.append({
            "xt": xt,
            "w3": np.ascontiguousarray(w3_c).astype(_BF16),
            "wp": np.ascontiguousarray(w_proj[FPC * c:FPC * (c + 1), :]).astype(_BF16),
            "masks": masks,
        })
    return in_maps


def unshard(results):
    total = results[0]["out"].astype(np.float32)
    for r in results[1:]:
        total += r["out"]
    return np.ascontiguousarray(total.transpose(0, 2, 1))


def run(inputs, trace=False, **kw):
    from concourse.bass_utils import run_bass_kernel_spmd

    nc = _get_nc()
    in_maps = shard_inputs(inputs["x"], inputs["w_qkv"], inputs["w_proj"])
    res = run_bass_kernel_spmd(nc, in_maps, core_ids=list(range(N_CORES)),
                               trace=trace, **kw)
    return unshard(res.results), res


def kernel(**inputs):
    out, _ = run(inputs, trace=False)
    return out


# revision 9
# speedup vs baseline: 1.3292x; 1.0046x over previous
"""Multi-head causal attention (B=4, T=2048, D=1024, H=16) on 8 TRN2 cores.

Tensor-parallel over heads: core c computes heads {2c, 2c+1}. Each core:
  - Q', K', V' feature-major ([feat, tok]) via 512-wide PE matmuls,
  - V' -> token-major V via PE transposes, augmented with a ones column
    (fused softmax denominator),
  - S^T = K'^T Q' tiles [128 k x 512 q] (so softmax reductions over keys
    happen on the PE and P^T is directly the PV moving operand),
  - exp (no max subtraction: |S|/32 <= ~2), multiplicative 0/1 causal mask,
    diagonal blocks narrowed to their live query range,
  - per-query normalization via DVE reciprocal + GpSimd partition broadcast,
  - w_proj row-slice partial matmul, written feature-major.
Host sums the 8 partial projections and transposes back.
"""

import sys

for _p in ("/opt/trn_rl_repo",):
    if _p not in sys.path:
        sys.path.append(_p)

import numpy as np
import ml_dtypes

B, T, D = 4, 2048, 1024
H = 16
HD = D // H
NORM = float(np.sqrt(D))
N_CORES = 8
HEADS_PER_CORE = H // N_CORES          # 2
FPC = HEADS_PER_CORE * HD              # 128 features per core
QC = 512                               # query chunk
NQC = T // QC                          # 4
KB = 128                               # key block
DKC = D // 128                         # 8 contraction chunks over D

_BF16 = ml_dtypes.bfloat16

_cache = {}


def _build():
    import concourse.bacc as bacc
    import concourse.mybir as mybir
    from concourse.tile import TileContext
    from concourse.alu_op_type import AluOpType
    from concourse.masks import make_identity

    f32 = mybir.dt.float32
    bf16 = mybir.dt.bfloat16
    EXP = mybir.ActivationFunctionType.Exp
    LN = mybir.ActivationFunctionType.Ln

    nc = bacc.Bacc("TRN2", target_bir_lowering=False, debug=False,
                   num_devices=N_CORES)

    xt = nc.dram_tensor("xt", [B, D, T], bf16, kind="ExternalInput").ap()
    w3 = nc.dram_tensor("w3", [D, 3 * FPC], bf16, kind="ExternalInput").ap()
    wp = nc.dram_tensor("wp", [FPC, D], bf16, kind="ExternalInput").ap()
    masks = nc.dram_tensor("masks", [4, KB, QC], bf16, kind="ExternalInput").ap()
    out = nc.dram_tensor("out", [B, D, T], f32, kind="ExternalOutput").ap()

    with TileContext(nc) as tc:
        with (
            tc.tile_pool(name="const", bufs=1) as cpool,
            tc.tile_pool(name="xp", bufs=16) as xpool,
            tc.tile_pool(name="qk", bufs=2) as qkpool,
            tc.tile_pool(name="vaug", bufs=20) as vpool,
            tc.tile_pool(name="pt", bufs=36) as ptpool,
            tc.tile_pool(name="y", bufs=2) as ypool,
            tc.tile_pool(name="sm", bufs=4) as smpool,
            tc.tile_pool(name="ot", bufs=4) as otpool,
            tc.tile_pool(name="psA", bufs=3, space="PSUM") as psA,
            tc.tile_pool(name="psB", bufs=1, space="PSUM") as psB,
            tc.tile_pool(name="psY", bufs=2, space="PSUM") as psY,
            tc.tile_pool(name="psO", bufs=2, space="PSUM") as psO,
        ):
            # ---- constants ----
            w3_t = []
            for kc in range(DKC):
                t = cpool.tile([128, 3 * FPC], bf16, tag=f"w3{kc}")
                nc.sync.dma_start(t[:], w3[kc * 128:(kc + 1) * 128, :])
                w3_t.append(t)
            wp_t = cpool.tile([FPC, D], bf16, tag="wp")
            nc.sync.dma_start(wp_t[:], wp[:])
            mask_t = []
            for p in range(4):
                t = cpool.tile([KB, QC], bf16, tag=f"mask{p}")
                nc.sync.dma_start(t[:], masks[p])
                mask_t.append(t)
            ones_col = cpool.tile([128, 1], bf16, tag="ones_col")
            nc.vector.memset(ones_col[:], 1.0)
            ident = cpool.tile([128, 128], bf16, tag="ident")
            make_identity(nc, ident[:])

            for b in range(B):
                # ---- load x^T (feature-major) ----
                xp_t = []
                for kc in range(DKC):
                    t = xpool.tile([128, T], bf16, tag="xp")
                    nc.sync.dma_start(t[:], xt[b, kc * 128:(kc + 1) * 128, :])
                    xp_t.append(t)

                # ---- Q', K', V' feature-major [128, T] ----
                with nc.named_scope("qkv"):
                    qp = qkpool.tile([128, T], bf16, tag="qp")
                    kp = qkpool.tile([128, T], bf16, tag="kp")
                    vp = qkpool.tile([128, T], bf16, tag="vp")
                    for ft, dst in ((0, qp), (1, kp), (2, vp)):
                        for ntk in range(NQC):
                            ps = psA.tile([128, QC], f32, tag="ps")
                            for kc in range(DKC):
                                nc.tensor.matmul(
                                    ps[:],
                                    lhsT=w3_t[kc][:, 128 * ft:128 * (ft + 1)],
                                    rhs=xp_t[kc][:, QC * ntk:QC * (ntk + 1)],
                                    start=(kc == 0), stop=(kc == DKC - 1),
                                )
                            nc.vector.tensor_copy(dst[:, QC * ntk:QC * (ntk + 1)], ps[:])

                # ---- V' -> token-major V, augmented with ones cols ----
                # layout: [v_h0(64) | ones | v_h1(64) | ones]
                with nc.named_scope("vtrans"):
                    vaug_t = []
                    for tk in range(T // 128):
                        ps = psB.tile([128, FPC], bf16, tag="psv")
                        nc.tensor.transpose(
                            ps[:], vp[:, 128 * tk:128 * (tk + 1)], ident[:]
                        )
                        va = vpool.tile([128, 2 * HD + 2], bf16, tag="vaug")
                        nc.vector.tensor_copy(va[:, 0:HD], ps[:, 0:HD])
                        nc.vector.tensor_copy(va[:, HD + 1:2 * HD + 1], ps[:, HD:2 * HD])
                        nc.gpsimd.memset(va[:, HD:HD + 1], 1.0)
                        nc.gpsimd.memset(va[:, 2 * HD + 1:2 * HD + 2], 1.0)
                        vaug_t.append(va)

                # ---- attention + projection per query chunk ----
                for qc in range(NQC):
                    nkb = (qc + 1) * (QC // KB)
                    y = ypool.tile([FPC, QC], bf16, tag="y")
                    pts = {}
                    with nc.named_scope("score"):
                        for h in range(HEADS_PER_CORE):
                            for kb in range(nkb):
                                p = kb - qc * 4
                                j0 = KB * p if p > 0 else 0
                                pss = psA.tile([128, QC], f32, tag="ps")
                                nc.tensor.matmul(
                                    pss[:, j0:QC],
                                    lhsT=kp[HD * h:HD * (h + 1), KB * kb:KB * (kb + 1)],
                                    rhs=qp[HD * h:HD * (h + 1), QC * qc + j0:QC * (qc + 1)],
                                    start=True, stop=True,
                                )
                                pt = ptpool.tile([KB, QC], bf16, tag="pt")
                                nc.scalar.activation(pt[:, j0:QC], pss[:, j0:QC],
                                                     EXP, scale=1.0 / NORM)
                                if p >= 0:
                                    nc.vector.tensor_tensor(
                                        pt[:, j0:QC], pt[:, j0:QC],
                                        mask_t[p][:, j0:QC],
                                        op=AluOpType.mult,
                                    )
                                pts[h, kb] = (pt, j0)
                    with nc.named_scope("pv"):
                        for h in range(HEADS_PER_CORE):
                            psy = psY.tile([HD + 1, QC], f32, tag="psy")
                            for kb in range(nkb):
                                pt, j0 = pts[h, kb]
                                nc.tensor.matmul(
                                    psy[0:HD + 1, j0:QC],
                                    lhsT=vaug_t[kb][:, (HD + 1) * h:(HD + 1) * (h + 1)],
                                    rhs=pt[:, j0:QC],
                                    start=(kb == 0), stop=(kb == nkb - 1),
                                )
                            # 1/sum as Exp(-Ln(sum)) on ACT: keeps the slow
                            # iterative divide off the in-order DVE queue.
                            lsum = smpool.tile([1, QC], f32, tag="lsum")
                            nc.scalar.activation(lsum[:], psy[HD:HD + 1, :], LN)
                            recip = smpool.tile([1, QC], f32, tag="recip")
                            nc.scalar.activation(recip[:], lsum[:], EXP, scale=-1.0)
                            bc = smpool.tile([64, QC], f32, tag="bc")
                            nc.gpsimd.partition_broadcast(bc[:], recip[:])
                            nc.vector.tensor_tensor(
                                y[HD * h:HD * (h + 1), :], psy[0:HD, :], bc[:],
                                op=AluOpType.mult,
                            )
                    with nc.named_scope("proj"):
                        for mt in range(D // 128):
                            pso = psO.tile([128, QC], f32, tag="pso")
                            nc.tensor.matmul(
                                pso[:],
                                lhsT=wp_t[:, 128 * mt:128 * (mt + 1)],
                                rhs=y[:],
                                start=True, stop=True,
                            )
                            ot = otpool.tile([128, QC], f32, tag="ot")
                            nc.vector.tensor_copy(ot[:], pso[:])
                            nc.sync.dma_start(
                                out[b, 128 * mt:128 * (mt + 1), QC * qc:QC * (qc + 1)],
                                ot[:],
                            )

    nc.compile()
    return nc


def _get_nc():
    if "nc" not in _cache:
        _cache["nc"] = _build()
    return _cache["nc"]


def _make_masks():
    i = np.arange(KB)[:, None]
    j = np.arange(QC)[None, :]
    m = np.zeros((4, KB, QC), dtype=np.float32)
    for p in range(4):
        m[p] = (j >= (KB * p + i)).astype(np.float32)
    return m.astype(_BF16)


def shard_inputs(x, w_qkv, w_proj):
    xt = np.ascontiguousarray(np.asarray(x, dtype=np.float32).transpose(0, 2, 1))
    xt = xt.astype(_BF16)
    w_qkv = np.asarray(w_qkv, dtype=np.float32)
    w_proj = np.asarray(w_proj, dtype=np.float32)
    masks = _make_masks()
    in_maps = []
    for c in range(N_CORES):
        qcols = slice(FPC * c, FPC * (c + 1))
        kcols = slice(D + FPC * c, D + FPC * (c + 1))
        vcols = slice(2 * D + FPC * c, 2 * D + FPC * (c + 1))
        w3_c = np.concatenate(
            [w_qkv[:, qcols], w_qkv[:, kcols], w_qkv[:, vcols]], axis=1)
        in_maps.append({
            "xt": xt,
            "w3": np.ascontiguousarray(w3_c).astype(_BF16),
            "wp": np.ascontiguousarray(w_proj[FPC * c:FPC * (c + 1), :]).astype(_BF16),
            "masks": masks,
        })
    return in_maps


def unshard(results):
    total = results[0]["out"].astype(np.float32)
    for r in results[1:]:
        total += r["out"]
    return np.ascontiguousarray(total.transpose(0, 2, 1))


def run(inputs, trace=False, **kw):
    from concourse.bass_utils import run_bass_kernel_spmd

    nc = _get_nc()
    in_maps = shard_inputs(inputs["x"], inputs["w_qkv"], inputs["w_proj"])
    res = run_bass_kernel_spmd(nc, in_maps, core_ids=list(range(N_CORES)),
                               trace=trace, **kw)
    return unshard(res.results), res


def kernel(**inputs):
    out, _ = run(inputs, trace=False)
    return out
